# revision 1
# baseline (speedup 1.0000x reference)
"""Causal multi-head attention on 8 trn2 NeuronCores.

Sharding: core c handles batch b=c//4 and heads [4*(c%4), 4*(c%4)+4).
Each core computes its 4 heads' attention plus the partial output
projection against the matching 256 rows of Wo; the host sums the 4
partials per batch (the all-reduce implied by row-sharding Wo).

Layout strategy (all fp32):
  - X^T [D,S] in SBUF so every matmul contracts d on partitions.
  - Q^T/K^T per head-pair [128, S] (two heads stacked on partitions),
    biases + 1/sqrt(dk) folded into the PSUM->SBUF eviction.
  - Scores computed transposed: ST[kv, q] = K^T.T @ Q^T. Causal mask is
    added in PSUM via an identity-weight matmul of a constant tile.
  - P = exp(ST) on ACT. Context ctxT[dk, q] = Vaug.T @ P where Vaug
    carries a ones column, so the softmax denominator lands in a spare
    PSUM partition of the same accumulation. V bias folds in post-norm
    because softmax rows sum to one.
  - Normalization: reciprocal of the denominator row, broadcast across
    partitions with a rank-1 PE matmul, one DVE multiply + bias add.
  - Output projection accumulates both 128-row chunks of ctx_cat^T plus
    a rank-1 bias matmul (bo only on cores with head-group 0).
"""

import sys

for _p in ("/opt/trn_rl_repo", "/root/.axon_site/_ro/trn_rl_repo"):
    if _p not in sys.path:
        sys.path.insert(0, _p)

import numpy as np

import concourse.bass as bass
import concourse.bacc as bacc
import concourse.tile as tile
from concourse import mybir
from concourse.bass_utils import run_bass_kernel_spmd

F32 = mybir.dt.float32
F32R = mybir.dt.float32r
BF16 = mybir.dt.bfloat16


def _r(ap):
    return ap.bitcast(F32R)

B, S, D, H, DK = 2, 2048, 1024, 16, 64
NCORES = 8
HPC = 4          # heads per core
NPAIR = 2        # head pairs per core
ND = D // 128    # 8 contraction chunks over d
NS = S // 512    # 4 query blocks
NS16 = S // 128  # 16 sequence chunks

_CACHE = {}


def _build_bass():
    nc = bacc.Bacc(None)
    xt = nc.dram_tensor("xt", [D, S],F32R, kind="ExternalInput")
    wq = nc.dram_tensor("wq", [NPAIR, D, 128],F32R, kind="ExternalInput")
    wk = nc.dram_tensor("wk", [NPAIR, D, 128],F32R, kind="ExternalInput")
    wv = nc.dram_tensor("wv", [D, 256],F32R, kind="ExternalInput")
    wo = nc.dram_tensor("wo", [256, D],F32R, kind="ExternalInput")
    bq = nc.dram_tensor("bq", [128, NPAIR], F32, kind="ExternalInput")
    bk = nc.dram_tensor("bk", [128, NPAIR], F32, kind="ExternalInput")
    bv = nc.dram_tensor("bv", [128, HPC], F32, kind="ExternalInput")
    mneg = nc.dram_tensor("mneg", [128, 4, 512], BF16, kind="ExternalInput")
    ident = nc.dram_tensor("ident", [128, 128], BF16, kind="ExternalInput")
    ones = nc.dram_tensor("ones", [1, 128],F32R, kind="ExternalInput")
    oneshalf = nc.dram_tensor("oneshalf", [1, 128],F32R, kind="ExternalInput")
    vfix = nc.dram_tensor("vfix", [128, 64], F32R, kind="ExternalInput")
    out = nc.dram_tensor("out", [S, D], F32, kind="ExternalOutput")

    with nc.allow_low_precision("fp32r operands; accumulation stays fp32 in PSUM"), \
            tile.TileContext(nc) as tc:
        with (
            tc.tile_pool(name="consts", bufs=1) as consts,
            tc.tile_pool(name="qkv", bufs=1) as qkv,
        ):
            wq_sb = consts.tile([128, NPAIR, ND, 128], F32R, tag="wq")
            wk_sb = consts.tile([128, NPAIR, ND, 128], F32R, tag="wk")
            wv_sb = consts.tile([128, ND, 256], F32R, tag="wv")
            wo_sb = consts.tile([128, 2, D], F32R, tag="wo")
            bq_sb = consts.tile([128, NPAIR], F32, tag="bq")
            bk_sb = consts.tile([128, NPAIR], F32, tag="bk")
            bv_sb = consts.tile([128, HPC], F32, tag="bv")
            mneg_sb = consts.tile([128, 4, 512], BF16, tag="mneg")
            ident_sb = consts.tile([128, 128], BF16, tag="ident")
            ones_sb = consts.tile([1, 128], F32R, tag="ones")
            oneshalf_sb = consts.tile([1, 128], F32R, tag="oneshalf")

            qt_sb = qkv.tile([128, NPAIR, S], F32R, tag="qt")
            kt_sb = qkv.tile([128, NPAIR, S], F32R, tag="kt")
            # Vaug per pair: cols 0:64 V_even | 64 ones | 65:128 zeros
            # | 128:192 V_odd. Even lhsT = cols 0:65 -> ctx on parts
            # 0:64 (+denominator row 64); odd lhsT = cols 64:192 ->
            # denominator on part 0, ctx on parts 64:128.
            va_sb = qkv.tile([128, NPAIR, NS16, 192], F32R, tag="va")
            ctxcat_sb = qkv.tile([128, 2, S], F32R, tag="ctxcat")

            # small consts first (tiny transfers, needed early)
            nc.sync.dma_start(out=bq_sb[:], in_=bq[:])
            nc.sync.dma_start(out=bk_sb[:], in_=bk[:])
            nc.sync.dma_start(out=bv_sb[:], in_=bv[:])
            nc.sync.dma_start(out=ident_sb[:], in_=ident[:])
            nc.sync.dma_start(out=ones_sb[:], in_=ones[:])
            nc.sync.dma_start(out=oneshalf_sb[:], in_=oneshalf[:])
            for p in range(NPAIR):
                vfix_bc = bass.AP(
                    tensor=vfix.ap().tensor,
                    offset=0,
                    ap=[[64, 128], [0, NS16], [1, 64]],
                )
                nc.gpsimd.dma_start(out=va_sb[:, p, :, 64:128], in_=vfix_bc)

            with (
                tc.tile_pool(name="xp", bufs=1) as xp,
                tc.tile_pool(name="mmp", bufs=6, space="PSUM") as mmp,
            ):
                xt_sb = xp.tile([128, ND, S], F32R, tag="xt")
                # interleave xt chunks with the weights that consume them
                # so the first projection matmuls start ~5us in
                for c in range(ND):
                    nc.sync.dma_start(
                        out=xt_sb[:, c, :], in_=xt[c * 128:(c + 1) * 128, :]
                    )
                    for p in range(NPAIR):
                        nc.sync.dma_start(
                            out=wq_sb[:, p, c, :],
                            in_=wq[p, c * 128:(c + 1) * 128, :],
                        )
                        nc.sync.dma_start(
                            out=wk_sb[:, p, c, :],
                            in_=wk[p, c * 128:(c + 1) * 128, :],
                        )
                    nc.sync.dma_start(
                        out=wv_sb[:, c, :], in_=wv[c * 128:(c + 1) * 128, :]
                    )
                nc.sync.dma_start(out=mneg_sb[:], in_=mneg[:])
                for k in range(2):
                    nc.sync.dma_start(
                        out=wo_sb[:, k, :], in_=wo[k * 128:(k + 1) * 128, :]
                    )

                # ---- Q^T / K^T projections (per pair, dk on partitions)
                for p in range(NPAIR):
                    for sb in range(NS):
                        qp = mmp.tile([128, 512], F32, tag="mm", name="qp")
                        for c in range(ND):
                            nc.tensor.matmul(
                                qp[:],
                                lhsT=wq_sb[:, p, c, :],
                                rhs=xt_sb[:, c, sb * 512:(sb + 1) * 512],
                                start=(c == 0),
                                stop=(c == ND - 1),
                            )
                        nc.scalar.activation(
                            out=qt_sb[:, p, sb * 512:(sb + 1) * 512],
                            in_=qp[:],
                            func=mybir.ActivationFunctionType.Identity,
                            bias=bq_sb[:, p:p + 1],
                            scale=0.125,
                        )
                        kp = mmp.tile([128, 512], F32, tag="mm", name="kp")
                        for c in range(ND):
                            nc.tensor.matmul(
                                kp[:],
                                lhsT=wk_sb[:, p, c, :],
                                rhs=xt_sb[:, c, sb * 512:(sb + 1) * 512],
                                start=(c == 0),
                                stop=(c == ND - 1),
                            )
                        nc.scalar.activation(
                            out=kt_sb[:, p, sb * 512:(sb + 1) * 512],
                            in_=kp[:],
                            func=mybir.ActivationFunctionType.Identity,
                            bias=bk_sb[:, p:p + 1],
                            scale=1.0,
                        )

                # ---- V in natural layout [s, dk], 4 heads at once
                for s16 in range(NS16):
                    vp = mmp.tile([128, 256], F32, tag="mm", name="vp")
                    for c in range(ND):
                        nc.tensor.matmul(
                            vp[:],
                            lhsT=xt_sb[:, c, s16 * 128:(s16 + 1) * 128],
                            rhs=wv_sb[:, c, :],
                            start=(c == 0),
                            stop=(c == ND - 1),
                        )
                    for h in range(HPC):
                        p, j = h // 2, h % 2
                        dst0 = 0 if j == 0 else 128
                        nc.vector.tensor_copy(
                            out=va_sb[:, p, s16, dst0:dst0 + 64],
                            in_=vp[:, h * 64:(h + 1) * 64],
                        )

            # ---- attention + output projection, per query block
            with (
                tc.tile_pool(name="stp", bufs=4, space="PSUM") as stp,
                tc.tile_pool(name="ctxp", bufs=2, space="PSUM") as ctxp,
                tc.tile_pool(name="ptp", bufs=6) as ptp,
                tc.tile_pool(name="smp", bufs=3) as smp,
                tc.tile_pool(name="outp", bufs=3) as outp,
            ):
                def emit_norm(ctx_ps, even, p, qb, h):
                    # normalization + bias, partition-aligned per parity.
                    # Emitted one head late so the PE stream has score/ctx
                    # work in flight while DVE/ACT turn the denominator
                    # into a broadcast reciprocal.
                    cs = 64 if even else 0
                    lo = 0 if even else 64
                    r = smp.tile([1, 512], F32R, tag="r", name="r")
                    nc.vector.reciprocal(out=r[:], in_=ctx_ps[cs:cs + 1, :])
                    bc_ps = stp.tile([128, 512], F32, tag="st", name="bc_ps")
                    if even:
                        nc.tensor.matmul(
                            bc_ps[0:64, :],
                            lhsT=ones_sb[0:1, 0:64],
                            rhs=r[:],
                            start=True,
                            stop=True,
                        )
                    else:
                        nc.tensor.matmul(
                            bc_ps[:],
                            lhsT=oneshalf_sb[0:1, :],
                            rhs=r[:],
                            start=True,
                            stop=True,
                        )
                    bc_sb = smp.tile([128, 512], F32, tag="bc", name="bc_sb")
                    nc.vector.tensor_copy(
                        out=bc_sb[lo:lo + 64, :], in_=bc_ps[lo:lo + 64, :]
                    )
                    tn = smp.tile([128, 512], F32, tag="tn", name="tn")
                    nc.vector.tensor_mul(
                        out=tn[lo:lo + 64, :],
                        in0=ctx_ps[lo:lo + 64, :],
                        in1=bc_sb[lo:lo + 64, :],
                    )
                    nc.vector.tensor_scalar_add(
                        out=ctxcat_sb[lo:lo + 64, p, qb * 512:(qb + 1) * 512],
                        in0=tn[lo:lo + 64, :],
                        scalar1=bv_sb[lo:lo + 64, h:h + 1],
                    )

                def emit_outproj(qb):
                    for s16 in range(qb * 4, (qb + 1) * 4):
                        for do in range(2):
                            op = ctxp.tile([128, 512], F32, tag="op", name="op", bufs=2)
                            nc.tensor.matmul(
                                op[:],
                                lhsT=ctxcat_sb[:, 0, s16 * 128:(s16 + 1) * 128],
                                rhs=wo_sb[:, 0, do * 512:(do + 1) * 512],
                                start=True,
                                stop=False,
                            )
                            nc.tensor.matmul(
                                op[:],
                                lhsT=ctxcat_sb[:, 1, s16 * 128:(s16 + 1) * 128],
                                rhs=wo_sb[:, 1, do * 512:(do + 1) * 512],
                                start=False,
                                stop=True,
                            )
                            ot = outp.tile([128, 512], F32, tag="ot", name="ot")
                            if do == 0:
                                nc.scalar.copy(out=ot[:], in_=op[:])
                            else:
                                nc.vector.tensor_copy(out=ot[:], in_=op[:])
                            nc.sync.dma_start(
                                out=out[s16 * 128:(s16 + 1) * 128,
                                        do * 512:(do + 1) * 512],
                                in_=ot[:],
                            )

                pending = None
                for qb in range(NS):
                    nch = (qb + 1) * 4
                    for h in range(HPC):
                        p, j = h // 2, h % 2
                        even = j == 0
                        qs = qt_sb[j * 64:(j + 1) * 64, p, qb * 512:(qb + 1) * 512]
                        ctx_ps = ctxp.tile([128, 512], F32, tag="ctx", name="ctx_ps")
                        ctx_out = ctx_ps[0:65, :] if even else ctx_ps[:]
                        for c in range(nch):
                            st = stp.tile([128, 512], F32, tag="st", name="st")
                            diag = c >= qb * 4
                            # columns [0, f0) of this block are fully masked
                            # (q < kv for all partitions): skip them entirely.
                            f0 = 128 * (c - qb * 4) if diag else 0
                            nc.tensor.matmul(
                                st[:, f0:512],
                                lhsT=kt_sb[j * 64:(j + 1) * 64, p,
                                           c * 128:(c + 1) * 128],
                                rhs=qs[:, f0:512],
                                start=True,
                                stop=not diag,
                            )
                            if diag:
                                nc.tensor.matmul(
                                    st[:, f0:512],
                                    lhsT=ident_sb[:],
                                    rhs=mneg_sb[:, c - qb * 4, f0:512],
                                    start=False,
                                    stop=True,
                                )
                            pt = ptp.tile([128, 512], F32R, tag="pt", name="pt")
                            nc.scalar.activation(
                                out=pt[:, f0:512],
                                in_=st[:, f0:512],
                                func=mybir.ActivationFunctionType.Exp,
                            )
                            lhsT_v = (
                                va_sb[:, p, c, 0:65]
                                if even
                                else va_sb[:, p, c, 64:192]
                            )
                            nc.tensor.matmul(
                                ctx_out[:, f0:512] if diag else ctx_out,
                                lhsT=lhsT_v,
                                rhs=pt[:, f0:512],
                                start=(c == 0),
                                stop=(c == nch - 1),
                            )
                        if pending is not None:
                            emit_norm(*pending)
                        pending = (ctx_ps, even, p, qb, h)
                        if h == 0 and qb > 0:
                            emit_outproj(qb - 1)

                emit_norm(*pending)
                pending = None
                emit_outproj(NS - 1)
    if not nc.is_finalized():
        nc.finalize()
    return nc


def _prep_inputs(embeddings, Wq, bq, Wk, bk, Wv, bv, Wo, bo):
    embeddings = np.asarray(embeddings, np.float32)
    Wq, bq = np.asarray(Wq, np.float32), np.asarray(bq, np.float32)
    Wk, bk = np.asarray(Wk, np.float32), np.asarray(bk, np.float32)
    Wv, bv = np.asarray(Wv, np.float32), np.asarray(bv, np.float32)
    Wo, bo = np.asarray(Wo, np.float32), np.asarray(bo, np.float32)

    import ml_dtypes
    bf16_t = ml_dtypes.bfloat16
    p_idx = np.arange(128)
    mneg = np.zeros((128, 4, 512), np.float32)
    for i in range(4):
        f = np.arange(512)[None, :]
        mneg[:, i, :] = np.where(f >= p_idx[:, None] + 128 * i, 0.0, -1e9)
    mneg = mneg.astype(bf16_t)
    ident = np.eye(128, dtype=np.float32).astype(bf16_t)
    vfix = np.zeros((128, 64), np.float32)
    vfix[:, 0] = 1.0
    ones = np.ones((1, 128), np.float32)
    oneshalf = np.concatenate(
        [np.zeros((1, 64), np.float32), np.ones((1, 64), np.float32)], axis=1
    )

    in_maps = []
    for c in range(NCORES):
        b, g = c // 4, c % 4
        hs = HPC * g
        xt = np.ascontiguousarray(embeddings[b].T)
        wq2 = np.stack(
            [np.concatenate([Wq[hs + 2 * p], Wq[hs + 2 * p + 1]], axis=1)
             for p in range(NPAIR)]
        )
        wk2 = np.stack(
            [np.concatenate([Wk[hs + 2 * p], Wk[hs + 2 * p + 1]], axis=1)
             for p in range(NPAIR)]
        )
        wv4 = np.concatenate([Wv[hs + h] for h in range(HPC)], axis=1)
        wo4 = np.ascontiguousarray(Wo[hs * DK:(hs + HPC) * DK, :])
        bq2 = np.stack(
            [np.concatenate([bq[hs + 2 * p], bq[hs + 2 * p + 1]]) / 8.0
             for p in range(NPAIR)], axis=1
        )
        bk2 = np.stack(
            [np.concatenate([bk[hs + 2 * p], bk[hs + 2 * p + 1]])
             for p in range(NPAIR)], axis=1
        )
        bv4 = np.stack(
            [np.tile(bv[hs + h], 2) for h in range(HPC)], axis=1
        )
        in_maps.append({
            "xt": np.ascontiguousarray(xt),
            "wq": np.ascontiguousarray(wq2),
            "wk": np.ascontiguousarray(wk2),
            "wv": np.ascontiguousarray(wv4),
            "wo": wo4,
            "bq": np.ascontiguousarray(bq2),
            "bk": np.ascontiguousarray(bk2),
            "bv": np.ascontiguousarray(bv4),
            "mneg": mneg,
            "ident": ident,
            "ones": ones,
            "oneshalf": oneshalf,
            "vfix": vfix,
        })
    return in_maps


def kernel(embeddings, Wq, bq, Wk, bk, Wv, bv, Wo, bo, _trace=False, _trace_kw=None):
    if "nc" not in _CACHE:
        _CACHE["nc"] = _build_bass()
    nc = _CACHE["nc"]
    in_maps = _prep_inputs(embeddings, Wq, bq, Wk, bk, Wv, bv, Wo, bo)
    kw = dict(_trace_kw or {})
    res = run_bass_kernel_spmd(
        nc, in_maps, core_ids=list(range(NCORES)), trace=_trace, **kw
    )
    _CACHE["last_result"] = res
    bo32 = np.asarray(bo, np.float32)
    out = np.empty((B, S, D), np.float32)
    for b in range(B):
        acc = np.array(res.results[4 * b]["out"], np.float32, copy=True)
        for g in range(1, 4):
            acc += np.asarray(res.results[4 * b + g]["out"], np.float32)
        out[b] = acc + bo32
    return out



# revision 4
# speedup vs baseline: 1.6575x; 1.6575x over previous
"""Causal multi-head attention on 8 trn2 NeuronCores.

Sharding: core c handles batch b=c//4 and heads [4*(c%4), 4*(c%4)+4).
Each core computes its 4 heads' attention plus the partial output
projection against the matching 256 rows of Wo; the host sums the 4
partials per batch (the all-reduce implied by row-sharding Wo) and adds
bo.

v3 vs the fp32r baseline:
  - All matmul operands are bf16 (PSUM accumulation stays fp32). The
    fp32r path runs in fp32_mode=HIGH at ~3.5 cycles/col on HW; bf16
    runs at 1 cycle/col and halves LDWEIGHTS + DMA time.
  - K bias dropped entirely: softmax is shift-invariant per query row,
    and q.bk / bq.bk are constant over kv. Only bq.k survives, so bq is
    folded into the Q projection via a rank-1 (bq x ones) matmul in the
    same PSUM accumulation; 1/sqrt(dk) is folded into Wq/bq on the host.
  - V bias folded into the V projection via a rank-1 (ones x bv) matmul:
    with the softmax-denominator ones-column trick this is exact.
  - The ctx matmul for chunk c is emitted one chunk late so the PE never
    stalls on ACT's exp latency (score/mask of c+1 stream while exp(c)
    runs).
  - Evictions: Q + outproj lo-half on ACT, K/V/denominator + outproj
    hi-half on DVE (Pool/GpSimd physically cannot touch PSUM).
    Softmax normalization: denominator row -> bf16 SBUF, rank-1 PE
    broadcast, reciprocal_approx_fast + multiply on DVE.
"""

import sys

for _p in ("/opt/trn_rl_repo", "/root/.axon_site/_ro/trn_rl_repo"):
    if _p not in sys.path:
        sys.path.insert(0, _p)

import numpy as np

import concourse.bass as bass
import concourse.bacc as bacc
import concourse.tile as tile
from concourse import mybir
from concourse.bass_utils import run_bass_kernel_spmd

F32 = mybir.dt.float32
BF16 = mybir.dt.bfloat16

B, S, D, H, DK = 2, 2048, 1024, 16, 64
NCORES = 8
HPC = 4          # heads per core
NPAIR = 2        # head pairs per core
ND = D // 128    # 8 contraction chunks over d
NS = S // 512    # 4 query blocks
NS16 = S // 128  # 16 sequence chunks

_CACHE = {}


def _build_bass():
    nc = bacc.Bacc(None)
    xt = nc.dram_tensor("xt", [D, S], BF16, kind="ExternalInput")
    wq = nc.dram_tensor("wq", [NPAIR, D, 128], BF16, kind="ExternalInput")
    wk = nc.dram_tensor("wk", [NPAIR, D, 128], BF16, kind="ExternalInput")
    wv = nc.dram_tensor("wv", [D, 256], BF16, kind="ExternalInput")
    wo = nc.dram_tensor("wo", [256, D], BF16, kind="ExternalInput")
    bq = nc.dram_tensor("bq", [1, NPAIR, 128], BF16, kind="ExternalInput")
    bvrow = nc.dram_tensor("bvrow", [1, 256], BF16, kind="ExternalInput")
    mneg = nc.dram_tensor("mneg", [128, 4, 512], BF16, kind="ExternalInput")
    ident = nc.dram_tensor("ident", [128, 128], BF16, kind="ExternalInput")
    ones = nc.dram_tensor("ones", [1, 512], BF16, kind="ExternalInput")
    vfix = nc.dram_tensor("vfix", [128, 64], BF16, kind="ExternalInput")
    out = nc.dram_tensor("out", [S, D], F32, kind="ExternalOutput")

    with nc.allow_low_precision("bf16 operands; accumulation stays fp32 in PSUM"), \
            tile.TileContext(nc) as tc:
        with (
            tc.tile_pool(name="consts", bufs=1) as consts,
            tc.tile_pool(name="qkv", bufs=1) as qkv,
        ):
            wq_sb = consts.tile([128, NPAIR, ND, 128], BF16, tag="wq")
            wk_sb = consts.tile([128, NPAIR, ND, 128], BF16, tag="wk")
            wv_sb = consts.tile([128, ND, 256], BF16, tag="wv")
            wo_sb = consts.tile([128, 2, D], BF16, tag="wo")
            bq_sb = consts.tile([1, NPAIR, 128], BF16, tag="bq")
            bvrow_sb = consts.tile([1, 256], BF16, tag="bvrow")
            mneg_sb = consts.tile([128, 4, 512], BF16, tag="mneg")
            ident_sb = consts.tile([128, 128], BF16, tag="ident")
            ones_sb = consts.tile([1, 512], BF16, tag="ones")

            qt_sb = qkv.tile([128, NPAIR, S], BF16, tag="qt")
            kt_sb = qkv.tile([128, NPAIR, S], BF16, tag="kt")
            # Vaug per pair: cols 0:64 V_even | 64 ones | 65:128 zeros
            # | 128:192 V_odd. Even lhsT = cols 0:65 -> ctx on parts
            # 0:64 (+denominator row 64); odd lhsT = cols 64:192 ->
            # denominator on part 0, ctx on parts 64:128.
            va_sb = qkv.tile([128, NPAIR, NS16, 192], BF16, tag="va")
            ctxcat_sb = qkv.tile([128, 2, S], BF16, tag="ctxcat")

            # small consts first (tiny transfers, needed early)
            nc.sync.dma_start(out=bq_sb[:], in_=bq[:])
            nc.sync.dma_start(out=bvrow_sb[:], in_=bvrow[:])
            nc.sync.dma_start(out=ident_sb[:], in_=ident[:])
            nc.sync.dma_start(out=ones_sb[:], in_=ones[:])
            for p in range(NPAIR):
                vfix_bc = bass.AP(
                    tensor=vfix.ap().tensor,
                    offset=0,
                    ap=[[64, 128], [0, NS16], [1, 64]],
                )
                nc.gpsimd.dma_start(out=va_sb[:, p, :, 64:128], in_=vfix_bc)

            with (
                tc.tile_pool(name="xp", bufs=1) as xp,
                tc.tile_pool(name="mmp", bufs=6, space="PSUM") as mmp,
            ):
                xt_sb = xp.tile([128, ND, S], BF16, tag="xt")
                # interleave xt chunks with the weights that consume them
                # so the first projection matmuls start early
                for c in range(ND):
                    nc.sync.dma_start(
                        out=xt_sb[:, c, :], in_=xt[c * 128:(c + 1) * 128, :]
                    )
                    for p in range(NPAIR):
                        nc.sync.dma_start(
                            out=wq_sb[:, p, c, :],
                            in_=wq[p, c * 128:(c + 1) * 128, :],
                        )
                        nc.sync.dma_start(
                            out=wk_sb[:, p, c, :],
                            in_=wk[p, c * 128:(c + 1) * 128, :],
                        )
                    nc.sync.dma_start(
                        out=wv_sb[:, c, :], in_=wv[c * 128:(c + 1) * 128, :]
                    )
                nc.sync.dma_start(out=mneg_sb[:], in_=mneg[:])
                for k in range(2):
                    nc.sync.dma_start(
                        out=wo_sb[:, k, :], in_=wo[k * 128:(k + 1) * 128, :]
                    )

                # ---- Q^T / K^T projections (per pair, dk on partitions)
                for p in range(NPAIR):
                    for sb in range(NS):
                        qp = mmp.tile([128, 512], F32, tag="mm", name="qp")
                        # rank-1 bias first: clears the bank, adds bq
                        nc.tensor.matmul(
                            qp[:],
                            lhsT=bq_sb[0:1, p, :],
                            rhs=ones_sb[0:1, :],
                            start=True,
                            stop=False,
                        )
                        for c in range(ND):
                            nc.tensor.matmul(
                                qp[:],
                                lhsT=wq_sb[:, p, c, :],
                                rhs=xt_sb[:, c, sb * 512:(sb + 1) * 512],
                                start=False,
                                stop=(c == ND - 1),
                            )
                        nc.scalar.copy(
                            out=qt_sb[:, p, sb * 512:(sb + 1) * 512],
                            in_=qp[:],
                        )
                        kp = mmp.tile([128, 512], F32, tag="mm", name="kp")
                        for c in range(ND):
                            nc.tensor.matmul(
                                kp[:],
                                lhsT=wk_sb[:, p, c, :],
                                rhs=xt_sb[:, c, sb * 512:(sb + 1) * 512],
                                start=(c == 0),
                                stop=(c == ND - 1),
                            )
                        nc.vector.tensor_copy(
                            out=kt_sb[:, p, sb * 512:(sb + 1) * 512],
                            in_=kp[:],
                        )

                # ---- V in natural layout [s, dk], 4 heads at once,
                # with bv folded in via a rank-1 matmul
                for s16 in range(NS16):
                    vp = mmp.tile([128, 256], F32, tag="mm", name="vp")
                    for c in range(ND):
                        nc.tensor.matmul(
                            vp[:],
                            lhsT=xt_sb[:, c, s16 * 128:(s16 + 1) * 128],
                            rhs=wv_sb[:, c, :],
                            start=(c == 0),
                            stop=False,
                        )
                    nc.tensor.matmul(
                        vp[:],
                        lhsT=ones_sb[0:1, 0:128],
                        rhs=bvrow_sb[0:1, :],
                        start=False,
                        stop=True,
                    )
                    # V_even -> va cols 0:64, V_odd -> cols 128:192 in one
                    # two-segment copy per pair
                    for p in range(NPAIR):
                        d0 = va_sb[:, p, s16, 0:64]
                        dst = bass.AP(
                            tensor=d0.tensor, offset=d0.offset,
                            ap=[[d0.ap[0][0], 128], [128, 2], [1, 64]],
                        )
                        s0 = vp[:, p * 128:(p + 1) * 128]
                        src = bass.AP(
                            tensor=s0.tensor, offset=s0.offset,
                            ap=[[s0.ap[0][0], 128], [64, 2], [1, 64]],
                        )
                        nc.vector.tensor_copy(out=dst, in_=src)

            # ---- attention + output projection, per query block
            with (
                tc.tile_pool(name="stp", bufs=4, space="PSUM") as stp,
                tc.tile_pool(name="ctxp", bufs=2, space="PSUM") as ctxp,
                tc.tile_pool(name="ptp", bufs=6) as ptp,
                tc.tile_pool(name="smp", bufs=3) as smp,
                tc.tile_pool(name="outp", bufs=3) as outp,
            ):
                def emit_norm(ctx_ps, even, p, qb, h):
                    # normalization, partition-aligned per parity.
                    # Emitted one head late so the PE stream has score/ctx
                    # work in flight while DVE/PE turn the denominator
                    # into a broadcast reciprocal.
                    cs = 64 if even else 0
                    lo = 0 if even else 64
                    den = smp.tile([1, 512], BF16, tag="den", name="den")
                    nc.vector.tensor_copy(out=den[:], in_=ctx_ps[cs:cs + 1, :])
                    # broadcast to all 128 partitions: custom-DVE ops (and
                    # tile_position=(0,64) matmuls) misbehave on HW when
                    # based at partition 64, so keep everything at base 0.
                    bc_ps = stp.tile([128, 512], F32, tag="st", name="bc_ps")
                    nc.tensor.matmul(
                        bc_ps[:],
                        lhsT=ones_sb[0:1, 0:128],
                        rhs=den[:],
                        start=True,
                        stop=True,
                    )
                    rcp = smp.tile([128, 512], F32, tag="rcp", name="rcp")
                    nc.vector.reciprocal_approx_fast(
                        out=rcp[:], in_=bc_ps[:]
                    )
                    nc.vector.tensor_mul(
                        out=ctxcat_sb[lo:lo + 64, p, qb * 512:(qb + 1) * 512],
                        in0=ctx_ps[lo:lo + 64, :],
                        in1=rcp[lo:lo + 64, :],
                    )

                def emit_outproj(qb):
                    for s16 in range(qb * 4, (qb + 1) * 4):
                        for do in range(2):
                            op = ctxp.tile([128, 512], F32, tag="op", name="op", bufs=2)
                            nc.tensor.matmul(
                                op[:],
                                lhsT=ctxcat_sb[:, 0, s16 * 128:(s16 + 1) * 128],
                                rhs=wo_sb[:, 0, do * 512:(do + 1) * 512],
                                start=True,
                                stop=False,
                            )
                            nc.tensor.matmul(
                                op[:],
                                lhsT=ctxcat_sb[:, 1, s16 * 128:(s16 + 1) * 128],
                                rhs=wo_sb[:, 1, do * 512:(do + 1) * 512],
                                start=False,
                                stop=True,
                            )
                            ot = outp.tile([128, 512], F32, tag="ot", name="ot")
                            if do == 0:
                                nc.scalar.copy(out=ot[:], in_=op[:])
                            else:
                                nc.vector.tensor_copy(out=ot[:], in_=op[:])
                            nc.sync.dma_start(
                                out=out[s16 * 128:(s16 + 1) * 128,
                                        do * 512:(do + 1) * 512],
                                in_=ot[:],
                            )

                pending = None
                for qb in range(NS):
                    nch = (qb + 1) * 4
                    for h in range(HPC):
                        p, j = h // 2, h % 2
                        even = j == 0
                        qs = qt_sb[j * 64:(j + 1) * 64, p, qb * 512:(qb + 1) * 512]
                        ctx_ps = ctxp.tile([128, 512], F32, tag="ctx", name="ctx_ps")
                        ctx_out = ctx_ps[0:65, :] if even else ctx_ps[:]
                        lagged = None  # (pt, f0, diag, c) awaiting its ctx mm

                        def emit_ctx(lag):
                            pt, f0, diag, c = lag
                            lhsT_v = (
                                va_sb[:, p, c, 0:65]
                                if even
                                else va_sb[:, p, c, 64:192]
                            )
                            nc.tensor.matmul(
                                ctx_out[:, f0:512] if diag else ctx_out,
                                lhsT=lhsT_v,
                                rhs=pt[:, f0:512],
                                start=(c == 0),
                                stop=(c == nch - 1),
                            )

                        for c in range(nch):
                            st = stp.tile([128, 512], F32, tag="st", name="st")
                            diag = c >= qb * 4
                            # columns [0, f0) of this block are fully masked
                            # (q < kv for all partitions): skip them entirely.
                            f0 = 128 * (c - qb * 4) if diag else 0
                            nc.tensor.matmul(
                                st[:, f0:512],
                                lhsT=kt_sb[j * 64:(j + 1) * 64, p,
                                           c * 128:(c + 1) * 128],
                                rhs=qs[:, f0:512],
                                start=True,
                                stop=not diag,
                            )
                            if diag:
                                nc.tensor.matmul(
                                    st[:, f0:512],
                                    lhsT=ident_sb[:],
                                    rhs=mneg_sb[:, c - qb * 4, f0:512],
                                    start=False,
                                    stop=True,
                                )
                            pt = ptp.tile([128, 512], BF16, tag="pt", name="pt")
                            nc.scalar.activation(
                                out=pt[:, f0:512],
                                in_=st[:, f0:512],
                                func=mybir.ActivationFunctionType.Exp,
                            )
                            # ctx for the previous chunk: its exp has had a
                            # full chunk of PE work to complete, so the PE
                            # never stalls on ACT latency.
                            if lagged is not None:
                                emit_ctx(lagged)
                            lagged = (pt, f0, diag, c)
                        emit_ctx(lagged)

                        if pending is not None:
                            emit_norm(*pending)
                        pending = (ctx_ps, even, p, qb, h)
                        if h == 0 and qb > 0:
                            emit_outproj(qb - 1)

                emit_norm(*pending)
                pending = None
                emit_outproj(NS - 1)
    if not nc.is_finalized():
        nc.finalize()
    return nc


def _prep_inputs(embeddings, Wq, bq, Wk, bk, Wv, bv, Wo, bo):
    embeddings = np.asarray(embeddings, np.float32)
    Wq, bq = np.asarray(Wq, np.float32), np.asarray(bq, np.float32)
    Wk = np.asarray(Wk, np.float32)
    Wv, bv = np.asarray(Wv, np.float32), np.asarray(bv, np.float32)
    Wo = np.asarray(Wo, np.float32)

    import ml_dtypes
    bf16_t = ml_dtypes.bfloat16
    p_idx = np.arange(128)
    mneg = np.zeros((128, 4, 512), np.float32)
    for i in range(4):
        f = np.arange(512)[None, :]
        mneg[:, i, :] = np.where(f >= p_idx[:, None] + 128 * i, 0.0, -1e9)
    mneg = mneg.astype(bf16_t)
    ident = np.eye(128, dtype=np.float32).astype(bf16_t)
    vfix = np.zeros((128, 64), np.float32)
    vfix[:, 0] = 1.0
    vfix = vfix.astype(bf16_t)
    ones = np.ones((1, 512), np.float32).astype(bf16_t)

    in_maps = []
    for c in range(NCORES):
        b, g = c // 4, c % 4
        hs = HPC * g
        xt = np.ascontiguousarray(embeddings[b].T).astype(bf16_t)
        # 1/sqrt(dk) folded into Wq/bq (exact power of two)
        wq2 = np.stack(
            [np.concatenate([Wq[hs + 2 * p], Wq[hs + 2 * p + 1]], axis=1)
             for p in range(NPAIR)]
        ) * 0.125
        wk2 = np.stack(
            [np.concatenate([Wk[hs + 2 * p], Wk[hs + 2 * p + 1]], axis=1)
             for p in range(NPAIR)]
        )
        wv4 = np.concatenate([Wv[hs + h] for h in range(HPC)], axis=1)
        wo4 = np.ascontiguousarray(Wo[hs * DK:(hs + HPC) * DK, :]).astype(bf16_t)
        bq2 = np.stack(
            [np.concatenate([bq[hs + 2 * p], bq[hs + 2 * p + 1]]) / 8.0
             for p in range(NPAIR)]
        )[None, :, :]
        bv4 = np.concatenate([bv[hs + h] for h in range(HPC)])[None, :]
        in_maps.append({
            "xt": xt,
            "wq": np.ascontiguousarray(wq2).astype(bf16_t),
            "wk": np.ascontiguousarray(wk2).astype(bf16_t),
            "wv": np.ascontiguousarray(wv4).astype(bf16_t),
            "wo": wo4,
            "bq": np.ascontiguousarray(bq2).astype(bf16_t),
            "bvrow": np.ascontiguousarray(bv4).astype(bf16_t),
            "mneg": mneg,
            "ident": ident,
            "ones": ones,
            "vfix": vfix,
        })
    return in_maps


def kernel(embeddings, Wq, bq, Wk, bk, Wv, bv, Wo, bo, _trace=False, _trace_kw=None):
    if "nc" not in _CACHE:
        _CACHE["nc"] = _build_bass()
    nc = _CACHE["nc"]
    in_maps = _prep_inputs(embeddings, Wq, bq, Wk, bk, Wv, bv, Wo, bo)
    kw = dict(_trace_kw or {})
    res = run_bass_kernel_spmd(
        nc, in_maps, core_ids=list(range(NCORES)), trace=_trace, **kw
    )
    _CACHE["last_result"] = res
    bo32 = np.asarray(bo, np.float32)
    out = np.empty((B, S, D), np.float32)
    for b in range(B):
        acc = np.array(res.results[4 * b]["out"], np.float32, copy=True)
        for g in range(1, 4):
            acc += np.asarray(res.results[4 * b + g]["out"], np.float32)
        out[b] = acc + bo32
    return out
